# revision 1
# baseline (speedup 1.0000x reference)
"""Trainium2 Bass kernel for DifferentiableDLT (batched weighted-DLT homography fit).

Contract: kernel(**inputs) takes FULL inputs
    flow (64, 2, 320, 576) f32, mask (64, 1, 320, 576) f32, img_h, img_w
and returns the FULL output (64, 3, 3) f32.

Strategy (pure data parallel, 8 batches/core x 8 cores):
  The 1024 sample points form a fixed separable 32x32 grid, so bilinear
  sampling touches only 64 of 320 rows (32 pairs of adjacent rows) and 64 of
  576 columns.  Per core we:
    1. dma_gather the 64 needed rows (pair vectors for flow, single rows for
       mask) from HBM directly into a partition-optimal SBUF layout.
    2. Select the 64 needed columns with 9 uniform-stride-run copies (DVE).
    3. Bilinear lerp in y then x (DVE tensor ops with constant weight tiles).
    4. One PE transpose puts points on partitions / (tile, batch) on free.
    5. Hartley-normalize dst points; build weighted feature products D =
       [w, w*p, w*q, w*(p^2+q^2)]; the 24 moments that fill A^T A | A^T b are
       C^T @ D with C the constant source-point feature matrix (PE matmul).
    6. Assemble the 8x9 augmented normal equations per batch (batch on
       partitions) and solve with unpivoted Gauss-Jordan (SPD + eps*I).
    7. Denormalize H, sign/scale fix, support gate, DMA out (8,3,3).
"""

import dataclasses
import math
import numpy as np

import concourse.bass as bass
import concourse.bacc as bacc
import concourse.mybir as mybir
from concourse import tile, library_config
from concourse import bass_utils

F32 = mybir.dt.float32
I16 = mybir.dt.int16
ALU = mybir.AluOpType
ACTF = mybir.ActivationFunctionType

NCORES = 8
BPC = 8          # batches per core
HF, WF = 320, 576
NG = 32          # grid is NG x NG points
NPTS = NG * NG
EPS = 1e-6

# ---------------------------------------------------------------------------
# host-side constant computation
# ---------------------------------------------------------------------------


def _grid_1d(size, n):
    m = int(size * 0.05)
    return np.linspace(m, size - m - 1, n, dtype=np.float32)


def _segments(x0):
    """Maximal uniform-step segments (start, len, step) covering x0."""
    segs = []
    i = 0
    n = len(x0)
    while i < n:
        if i == n - 1:
            segs.append((i, 1, 1))
            break
        st = x0[i + 1] - x0[i]
        j = i + 1
        while j + 1 < n and x0[j + 1] - x0[j] == st:
            j += 1
        segs.append((i, j - i + 1, int(st)))
        i = j + 1
    return segs


def _wrap16(idxlist, nslots):
    """dma_gather/ap_gather index wrapping: list pos k -> partition k%16,
    slot k//16, replicated across the 8 gpsimd cores (16-partition groups)."""
    base = np.zeros((16, nslots), np.int16)
    for k, v in enumerate(idxlist):
        base[k % 16, k // 16] = v
    return np.tile(base, (8, 1))


class _Consts:
    def __init__(self, img_h, img_w):
        ys = _grid_1d(HF, NG)
        xs = _grid_1d(WF, NG)
        y0 = np.floor(ys).astype(np.int64)
        x0 = np.floor(xs).astype(np.int64)
        wy = (ys.astype(np.float64) - y0)
        wx = (xs.astype(np.float64) - x0)
        self.segs = _segments(x0)
        self.x0 = x0
        self.sx = float(np.float32((img_w - 1) / max(WF - 1, 1)))
        self.sy = float(np.float32((img_h - 1) / max(HF - 1, 1)))

        # grid points: n = j*NG + i -> (x=xs[i], y=ys[j])
        j = np.arange(NPTS) // NG
        i = np.arange(NPTS) % NG
        gx = xs.astype(np.float64)[i]
        gy = ys.astype(np.float64)[j]

        # constant Hartley normalization of the source points (image coords)
        sxi = gx * self.sx
        syi = gy * self.sy
        mx, my = sxi.mean(), syi.mean()
        cxs, cys = sxi - mx, syi - my
        s_src = max(np.sqrt(cxs * cxs + cys * cys).mean() / math.sqrt(2.0), 1e-8)
        u = cxs / s_src
        v = cys / s_src
        # T_src = [[1/s,0,-mx/s],[0,1/s,-my/s],[0,0,1]] immediates
        self.a_ts = float(np.float32(1.0 / s_src))
        self.c_ts = float(np.float32(-mx / s_src))
        self.d_ts = float(np.float32(-my / s_src))

        # ---- dma_gather index tables ----
        # flow: 512 pair vectors; v = s*128 + p, p = kd*16 + (c*8+b), k = kd*4+s
        self.giF = []
        for h in range(2):
            fidx = np.zeros(256, np.int64)
            for vv in range(256):
                s = 2 * h + vv // 128
                p = vv % 128
                kd, bc = p // 16, p % 16
                c, b = bc // 8, bc % 8
                k = kd * 4 + s
                fidx[vv] = (b * 2 + c) * HF + y0[k]
            self.giF.append(_wrap16(fidx, 16))
        # mask: 512 single-row vectors; p = a*64 + kd*8 + b, k = kd*4+s
        midx = np.zeros(512, np.int64)
        for vv in range(512):
            s = vv // 128
            p = vv % 128
            a = p // 64
            r = p % 64
            kd, b = r // 8, r % 8
            k = kd * 4 + s
            midx[vv] = b * HF + y0[k] + a
        self.giM = _wrap16(midx, 32)

        # ---- interp weight tiles ----
        # flow: partition p=(kd,bc'), free = s*64 + i*2 + c2 ; weight wy[kd*4+s]
        kd_p = np.arange(128) // 16
        s_f = np.arange(256) // 64
        self.WYF = np.asarray(
            wy[kd_p[:, None] * 4 + s_f[None, :]], np.float32
        )  # (128, 256)
        # mask: partition p=(kd,b) in [0,64), same free layout
        kd_m = np.arange(64) // 8
        self.WYM = np.asarray(wy[kd_m[:, None] * 4 + s_f[None, :]], np.float32)
        # x weights: free = s*32 + i
        i_f = np.arange(128) % 32
        self.WXF = np.tile(np.asarray(wx[i_f], np.float32)[None, :], (128, 1))

        # ---- point-feature matrix C6 (128, 8*6): F=[uu, uv, u, vv, v, 1] ----
        feats = np.stack([u * u, u * v, u, v * v, v, np.ones_like(u)], -1)  # (N,6)
        self.C6 = np.ascontiguousarray(
            feats.reshape(8, 128, 6).transpose(1, 0, 2).reshape(128, 48)
        ).astype(np.float32)

        # ---- fused transpose-scale + grid-offset matmul constants ----
        # psF[f, j] = sampF[j, f] * s(c(j)) + grid(f, j), with j = kd*16+c*8+b
        # grid(f, j) = c==0 ? xs[f%32]*sx : ys[4*kd + f//32]*sy
        jj = np.arange(128)
        kd_j = jj // 16
        c_j = (jj % 16) // 8
        sxy = np.where(c_j == 0, self.sx, self.sy).astype(np.float64)
        self.SXYD = (np.eye(128) * sxy[None, :]).astype(np.float32)
        ff = np.arange(128)
        G5 = np.zeros((5, 128), np.float64)
        GR5 = np.zeros((5, 128), np.float64)
        for sp in range(4):
            G5[sp] = (ff // 32 == sp).astype(np.float64)
            GR5[sp] = np.where(c_j == 1, ys.astype(np.float64)[4 * kd_j + sp] * self.sy, 0.0)
        G5[4] = xs.astype(np.float64)[ff % 32]
        GR5[4] = np.where(c_j == 0, self.sx, 0.0)
        self.G5 = G5.astype(np.float32)
        self.GR5 = GR5.astype(np.float32)

        self.IDN = np.eye(128, dtype=np.float32)

        # ---- E matrices: AUG[r*9+c] = sum_q sum_m EQ[q][m, r*9+c] * Mq[m] ----
        E = np.zeros((4, 6, 72), np.float64)
        sym = [[0, 1, 2], [1, 3, 4], [2, 4, 5]]
        for r in range(3):
            for c in range(3):
                m = sym[r][c]
                E[0, m, r * 9 + c] += 1
                E[0, m, (r + 3) * 9 + (c + 3)] += 1
        cr = [[0, 1], [1, 3], [2, 4]]
        for q, r0 in ((1, 0), (2, 3)):
            for r in range(3):
                for c2 in range(2):
                    m = cr[r][c2]
                    E[q, m, (r0 + r) * 9 + 6 + c2] += -1
                    E[q, m, (6 + c2) * 9 + (r0 + r)] += -1
            for r, m in ((0, 2), (1, 4), (2, 5)):
                E[q, m, (r0 + r) * 9 + 8] += 1
        rb = [[0, 1], [1, 3]]
        for r in range(2):
            for c2 in range(2):
                E[3, rb[r][c2], (6 + r) * 9 + 6 + c2] += 1
        E[3, 2, 6 * 9 + 8] += -1
        E[3, 4, 7 * 9 + 8] += -1
        self.EQ = np.ascontiguousarray(
            E.transpose(1, 0, 2).reshape(6, 288)
        ).astype(np.float32)


# ---------------------------------------------------------------------------
# device program
# ---------------------------------------------------------------------------


def _rows_view(ap, nrows, elem):
    """Overlapping rows view of a DRAM tensor: [(WF, nrows), (1, elem)]."""
    flat = ap.rearrange("b c h w -> (b c h w)").unsqueeze(0)
    return dataclasses.replace(flat, ap=[[WF, nrows], [1, elem]])


def _build_program(cc: _Consts):
    nc = bacc.Bacc("TRN2", target_bir_lowering=False, debug=False,
                   num_swdge_queues=2)

    flow = nc.dram_tensor("flow", [BPC, 2, HF, WF], F32, kind="ExternalInput")
    mask = nc.dram_tensor("mask", [BPC, 1, HF, WF], F32, kind="ExternalInput")
    giF0 = nc.dram_tensor("giF0", [128, 16], I16, kind="ExternalInput")
    giF1 = nc.dram_tensor("giF1", [128, 16], I16, kind="ExternalInput")
    giM = nc.dram_tensor("giM", [128, 32], I16, kind="ExternalInput")
    WYF = nc.dram_tensor("WYF", [128, 256], F32, kind="ExternalInput")
    WYM = nc.dram_tensor("WYM", [64, 256], F32, kind="ExternalInput")
    WXF = nc.dram_tensor("WXF", [128, 128], F32, kind="ExternalInput")
    C6 = nc.dram_tensor("C6", [128, 48], F32, kind="ExternalInput")
    SXYD = nc.dram_tensor("SXYD", [128, 128], F32, kind="ExternalInput")
    G5 = nc.dram_tensor("G5", [5, 128], F32, kind="ExternalInput")
    GR5 = nc.dram_tensor("GR5", [5, 128], F32, kind="ExternalInput")
    IDN = nc.dram_tensor("IDN", [128, 128], F32, kind="ExternalInput")
    EQ = nc.dram_tensor("EQ", [6, 288], F32, kind="ExternalInput")
    Hout = nc.dram_tensor("H", [BPC, 3, 3], F32, kind="ExternalOutput")

    V = nc.vector
    A = nc.scalar
    T = nc.tensor
    G = nc.gpsimd
    S = nc.sync

    with tile.TileContext(nc) as tc:
        with (
            tc.tile_pool(name="sb", bufs=1) as pool,
            tc.tile_pool(name="ps", bufs=1, space="PSUM") as psp,
        ):
            # ---------------- constants in ----------------
            def cin(name, src, shape, dtype=F32):
                t = pool.tile(list(shape), dtype, tag=name)
                S.dma_start(t[tuple(slice(0, s) for s in shape)], src[:])
                return t

            # index tables first, on the vector engine's HWDGE queue so the
            # gathers can launch without queueing behind the other constants
            giF0_t = pool.tile([128, 16], I16, tag="giF0")
            A.dma_start(giF0_t[:, :], giF0[:])
            giF1_t = pool.tile([128, 16], I16, tag="giF1")
            A.dma_start(giF1_t[:, :], giF1[:])
            giM_t = pool.tile([128, 32], I16, tag="giM")
            A.dma_start(giM_t[:, :], giM[:])
            WYF_t = cin("WYF", WYF, (128, 256))
            WYM_t = cin("WYM", WYM, (64, 256))
            WXF_t = cin("WXF", WXF, (128, 128))
            C6_t = cin("C6", C6, (128, 48))
            SXYD_t = cin("SXYD", SXYD, (128, 128))
            G5_t = cin("G5", G5, (5, 128))
            GR5_t = cin("GR5", GR5, (5, 128))
            IDN_t = cin("IDN", IDN, (128, 128))
            EQ_t = cin("EQ", EQ, (6, 288))
            IEYE_t = pool.tile([8, 9], F32, tag="IEYE")
            V.memset(IEYE_t[:, :], 0.0)
            V.memset(IEYE_t[:, 0:9:4], 1.0)
            ONESC_t = pool.tile([128, 1], F32, tag="ONESC")
            V.memset(ONESC_t[:, :], 1.0 / NPTS)
            ONESR_t = pool.tile([1, 128], F32, tag="ONESR")
            V.memset(ONESR_t[:, :], 1.0)
            # prefetch the ACT function table (Sqrt/Abs) off the critical path
            ACTJ = pool.tile([8, 2], F32, tag="ACTJ")
            V.memset(ACTJ[:, :], 1.0)
            A.activation(ACTJ[:, 0:1], ACTJ[:, 1:2], ACTF.Sqrt)
            A.activation(ACTJ[:, 1:2], ACTJ[:, 0:1], ACTF.Abs)
            # ---------------- row gathers (flow halves first) ----------------
            tF = pool.tile([128, 4, 1152], F32)   # [p=(kd,bc')][s][pair row]
            tM = pool.tile([128, 4, 576], F32)    # [p=(a,kd,b)][s][row]
            for h, gi_t in ((0, giF0_t), (1, giF1_t)):
                G.dma_gather(
                    out_ap=tF[:, 2 * h : 2 * h + 2, :],
                    in_ap=_rows_view(flow.ap(), 2 * BPC * HF - 2, 1152),
                    idxs_ap=gi_t[:, :],
                    num_idxs=256,
                    num_idxs_reg=256,
                    elem_size=1152,
                    elem_step=WF,
                    queue_num=0,
                )
            G.dma_gather(
                out_ap=tM[:, :, :],
                in_ap=_rows_view(mask.ap(), BPC * HF, 576),
                idxs_ap=giM_t[:, :],
                num_idxs=512,
                num_idxs_reg=512,
                elem_size=576,
                queue_num=1,
            )

            # ------- column select + bilinear interp, per flow half -------
            GxF = pool.tile([128, 2, 4, 32, 2], F32)  # [a][s][i][c2]
            GxM = pool.tile([128, 4, 32, 2], F32)     # [s][i][c2]
            tFv = tF[:, :, :].rearrange("p s (a w) -> p s a w", a=2)
            dF = pool.tile([128, 256], F32)
            VFt = pool.tile([128, 256], F32)
            dcF = pool.tile([128, 128], F32)
            sampF = pool.tile([128, 128], F32)
            for h in range(2):
                sl = slice(2 * h, 2 * h + 2)
                for (i0, L, st) in cc.segs:
                    base = int(cc.x0[i0])
                    for c2 in (0, 1):
                        src_ = tFv[:, sl, :, base + c2 : base + c2 + (L - 1) * st + 1 : st]
                        dst = GxF[:, :, sl, i0 : i0 + L, c2].transpose([0, 2, 1, 3])
                        V.tensor_copy(dst, src_)
                # rows: V = G0 + (G1-G0)*wy   (per-half views, 128 free)
                g0 = GxF[:, 0, sl, :, :].rearrange("p s i c -> p (s i c)")
                g1 = GxF[:, 1, sl, :, :].rearrange("p s i c -> p (s i c)")
                dh = dF[:, 128 * h : 128 * h + 128]
                vh = VFt[:, 128 * h : 128 * h + 128]
                wyh = WYF_t[:, 128 * h : 128 * h + 128]
                V.tensor_tensor(out=dh, in0=g1, in1=g0, op=ALU.subtract)
                V.tensor_tensor(out=dh, in0=dh, in1=wyh, op=ALU.mult)
                V.tensor_tensor(out=vh, in0=dh, in1=g0, op=ALU.add)
                # cols: samp = V0 + (V1-V0)*wx
                vv4 = vh.rearrange("p (s i c) -> p s i c", s=2, i=32, c=2)
                d3 = dcF[:, 64 * h : 64 * h + 64].rearrange("p (s i) -> p s i", s=2)
                s3 = sampF[:, 64 * h : 64 * h + 64].rearrange("p (s i) -> p s i", s=2)
                wx3 = WXF_t[:, 64 * h : 64 * h + 64].rearrange("p (s i) -> p s i", s=2)
                V.tensor_tensor(out=d3, in0=vv4[:, :, :, 1], in1=vv4[:, :, :, 0], op=ALU.subtract)
                V.tensor_tensor(out=d3, in0=d3, in1=wx3, op=ALU.mult)
                V.tensor_tensor(out=s3, in0=d3, in1=vv4[:, :, :, 0], op=ALU.add)
            # mask (single gather): select, then lerp across partition halves
            for (i0, L, st) in cc.segs:
                base = int(cc.x0[i0])
                for c2 in (0, 1):
                    srcm = tM[:, :, base + c2 : base + c2 + (L - 1) * st + 1 : st]
                    V.tensor_copy(GxM[:, :, i0 : i0 + L, c2], srcm)
            GxM2 = GxM[:, :, :, :].rearrange("p s i c -> p (s i c)")
            dM = pool.tile([64, 256], F32)
            VMt = pool.tile([64, 256], F32)
            GxMhi = pool.tile([64, 256], F32)
            V.tensor_copy(GxMhi[:, :], GxM2[64:128, :])
            V.tensor_tensor(out=dM[:, :], in0=GxMhi[:, :], in1=GxM2[0:64, :], op=ALU.subtract)
            V.tensor_tensor(out=dM[:, :], in0=dM[:, :], in1=WYM_t[:, :], op=ALU.mult)
            V.tensor_tensor(out=VMt[:, :], in0=dM[:, :], in1=GxM2[0:64, :], op=ALU.add)
            VMv = VMt[:, :].rearrange("p (s i c) -> p s i c", s=4, i=32, c=2)
            dcM = pool.tile([64, 128], F32)
            sampM = pool.tile([64, 128], F32)
            dm3 = dcM[:, :].rearrange("p (s i) -> p s i", s=4)
            sm3 = sampM[:, :].rearrange("p (s i) -> p s i", s=4)
            wxm = WXF_t[0:64, :].rearrange("p (s i) -> p s i", s=4)
            V.tensor_tensor(out=dm3, in0=VMv[:, :, :, 1], in1=VMv[:, :, :, 0], op=ALU.subtract)
            V.tensor_tensor(out=dm3, in0=dm3, in1=wxm, op=ALU.mult)
            V.tensor_tensor(out=sm3, in0=dm3, in1=VMv[:, :, :, 0], op=ALU.add)

            # ---- transpose to points-on-partitions, fused with image-coord
            # ---- scaling and grid offsets: PQs = samp^T * diag(sxy) + grid
            psF = psp.tile([128, 128], F32)
            psM = psp.tile([128, 64], F32)
            T.matmul(psF[:, :], sampF[:, :], SXYD_t[:, :], start=True, stop=False)
            T.matmul(psF[:, :], G5_t[:, :], GR5_t[:, :], start=False, stop=True)
            T.transpose(psM[:, :], sampM[:, :], IDN_t[0:64, 0:64])
            PQs = pool.tile([128, 128], F32)  # dst img coords [pl][t 8][c 2][b 8]
            SM = pool.tile([128, 64], F32)    # mask sample    [pl][t 8][b 8]
            V.tensor_copy(PQs[:, :], psF[:, :])
            V.tensor_copy(SM[:, :], psM[:, :])

            # ---------------- Hartley stats ----------------
            psSum = psp.tile([1, 128], F32, tag="pss")
            T.matmul(psSum[:, :], ONESC_t[:, :], PQs[:, :], start=True, stop=True)
            SRow = pool.tile([1, 128], F32)
            V.tensor_copy(SRow[:, :], psSum[:, :])
            MRow = pool.tile([1, 16], F32)   # [c 2][b 8] means
            V.tensor_reduce(
                out=MRow[:, :].rearrange("o (g b) -> o g b", g=2, b=8),
                in_=SRow[:, :].rearrange("o (t g b) -> o g b t", g=2, t=8, b=8),
                axis=mybir.AxisListType.X,
                op=ALU.add,
            )
            psMB = psp.tile([128, 16], F32, tag="pss")
            T.matmul(psMB[:, :], ONESR_t[:, :], MRow[:, :], start=True, stop=True)
            MB = pool.tile([128, 16], F32)
            V.tensor_copy(MB[:, :], psMB[:, :])

            CXY = pool.tile([128, 128], F32)  # centered dst [t][c][b]
            SQ = pool.tile([128, 128], F32)
            R2 = pool.tile([128, 64], F32)
            SQR = pool.tile([128, 64], F32)
            mbv = MB[:, :].rearrange("p (c b) -> p c b", c=2, b=8).unsqueeze(1)
            V.tensor_tensor(out=CXY[:, :].rearrange("p (t c b) -> p t c b", t=8, c=2, b=8),
                            in0=PQs[:, :].rearrange("p (t c b) -> p t c b", t=8, c=2, b=8),
                            in1=mbv.broadcast_to([128, 8, 2, 8]), op=ALU.subtract)
            V.tensor_tensor(out=SQ[:, :], in0=CXY[:, :], in1=CXY[:, :], op=ALU.mult)
            sq3 = SQ[:, :].rearrange("p (t c b) -> p c t b", t=8, c=2, b=8)
            V.tensor_tensor(out=R2[:, :].rearrange("p (t b) -> p t b", t=8),
                            in0=sq3[:, 0, :, :], in1=sq3[:, 1, :, :], op=ALU.add)
            A.activation(SQR[:, :], R2[:, :], ACTF.Sqrt)
            psSq = psp.tile([1, 64], F32, tag="pss")
            T.matmul(psSq[:, :], ONESC_t[:, :], SQR[:, :], start=True, stop=True)
            SqRow = pool.tile([1, 64], F32)
            V.tensor_copy(SqRow[:, :], psSq[:, :])
            sRow = pool.tile([1, 8], F32)
            V.tensor_reduce(
                out=sRow[:, :].unsqueeze(1),
                in_=SqRow[:, :].rearrange("o (t b) -> o b t", t=8),
                axis=mybir.AxisListType.X,
                op=ALU.add,
            )
            V.tensor_scalar(out=sRow[:, :], in0=sRow[:, :],
                            scalar1=1.0 / math.sqrt(2.0), op0=ALU.mult,
                            scalar2=1e-8, op1=ALU.max)
            IR24 = pool.tile([1, 24], F32)   # [inv | inv | inv^2]
            V.reciprocal(IR24[:, 0:8], sRow[:, :])
            V.tensor_copy(IR24[:, 8:16], IR24[:, 0:8])
            V.tensor_tensor(out=IR24[:, 16:24], in0=IR24[:, 0:8], in1=IR24[:, 0:8],
                            op=ALU.mult)

            # ---------------- D = [w, w*cx, w*cy, w*r2] (unnormalized) -----
            D = pool.tile([128, 256], F32)    # [pl][t 8][q 4][b 8]
            Dv = D[:, :].rearrange("p (t q b) -> p q t b", q=4, b=8)
            V.tensor_scalar(out=Dv[:, 0, :, :],
                            in0=SM[:, :].rearrange("p (t b) -> p t b", t=8),
                            scalar1=0.0, op0=ALU.max, scalar2=None)  # w
            d12 = D[:, :].rearrange("p (t q b) -> p t q b", q=4, b=8)[:, :, 1:3, :]
            cxy12 = CXY[:, :].rearrange("p (t c b) -> p t c b", t=8, c=2, b=8)
            wb2 = Dv[:, 0, :, :].unsqueeze(2).broadcast_to([128, 8, 2, 8])
            V.tensor_tensor(out=d12, in0=cxy12, in1=wb2, op=ALU.mult)
            V.tensor_tensor(out=Dv[:, 3, :, :],
                            in0=R2[:, :].rearrange("p (t b) -> p t b", t=8),
                            in1=Dv[:, 0, :, :], op=ALU.mult)

            # ---------------- moments: M = C^T D ----------------
            psMom = psp.tile([6, 32], F32)
            for t in range(8):
                T.matmul(psMom[:, :], C6_t[:, 6 * t : 6 * t + 6],
                         D[:, 32 * t : 32 * t + 32], start=(t == 0), stop=(t == 7))
            Msb = pool.tile([6, 32], F32)
            V.tensor_copy(Msb[:, :], psMom[:, :])
            # normalize the moment columns: [wp, wq] *= 1/s ; [wr] *= 1/s^2
            psC6 = psp.tile([6, 24], F32, tag="pss")
            T.matmul(psC6[:, :], ONESR_t[0:1, 0:6], IR24[:, :], start=True, stop=True)
            SC6 = pool.tile([6, 24], F32)
            V.tensor_copy(SC6[:, :], psC6[:, :])
            V.tensor_tensor(out=Msb[:, 8:32], in0=Msb[:, 8:32], in1=SC6[:, :],
                            op=ALU.mult)

            # ---------------- assemble [A^T A | A^T b] via PE ----------------
            psA = psp.tile([72, 8], F32)
            for q in range(4):
                T.matmul(psA[:, :], EQ_t[:, 72 * q : 72 * q + 72],
                         Msb[0:6, 8 * q : 8 * q + 8], start=(q == 0), stop=(q == 3))
            AS = pool.tile([72, 8], F32)
            V.tensor_copy(AS[:, :], psA[:, :])
            psAT = psp.tile([8, 72], F32, tag="pss")
            T.transpose(psAT[:, :], AS[:, :], IDN_t[0:72, 0:72])
            AUG = pool.tile([8, 72], F32)
            V.tensor_copy(AUG[:, :], psAT[:, :])
            V.tensor_scalar(out=AUG[:, 0:71:10], in0=AUG[:, 0:71:10],
                            scalar1=EPS, op0=ALU.add, scalar2=None)

            # ---------------- per-batch scalars to partitions --------------
            PR = pool.tile([1, 128], F32)
            V.tensor_copy(PR[:, 0:8], MRow[:, 0:8])
            V.tensor_copy(PR[:, 32:40], MRow[:, 8:16])
            V.tensor_copy(PR[:, 64:72], sRow[:, :])
            psSC = psp.tile([128, 1], F32, tag="pss")
            T.transpose(psSC[:, :], PR[:, :], IDN_t[0:1, 0:1])
            SC = pool.tile([128, 1], F32)
            V.tensor_copy(SC[:, :], psSC[:, :])
            SCC = pool.tile([8, 4], F32)
            V.tensor_copy(SCC[:, 0:1], SC[0:8, :])      # mx (dst mean x)
            V.tensor_copy(SCC[:, 1:2], SC[32:40, :])    # my
            V.tensor_copy(SCC[:, 2:3], SC[64:72, :])    # s_dst
            # support gate: AUG[2,2] = S1 + eps
            V.tensor_scalar(out=SCC[:, 3:4], in0=AUG[:, 20:21],
                            scalar1=NPTS * 1e-4 + EPS, op0=ALU.is_gt, scalar2=None)

            # ---------------- Gauss-Jordan ----------------
            RK = pool.tile([8, 9], F32)
            PIV = pool.tile([8, 1], F32)
            U8 = pool.tile([8, 72], F32)
            for k in range(8):
                w_ = 9 - k  # active columns k..8
                V.reciprocal(PIV[:, :], AUG[:, 9 * k + k : 9 * k + k + 1])
                V.tensor_scalar(out=RK[:, 0:w_], in0=AUG[:, 9 * k + k : 9 * k + 9],
                                scalar1=PIV[:, :], op0=ALU.mult, scalar2=None)
                fcol = AUG[:, k : 72 : 9].unsqueeze(2).broadcast_to([8, 8, w_])
                rkb = RK[:, 0:w_].unsqueeze(1).broadcast_to([8, 8, w_])
                ucols = U8[:, :].rearrange("p (r c) -> p r c", r=8)[:, :, 0:w_]
                acols = AUG[:, :].rearrange("p (r c) -> p r c", r=8)[:, :, k:9]
                V.tensor_tensor(out=ucols, in0=fcol, in1=rkb, op=ALU.mult)
                V.tensor_tensor(out=acols, in0=acols, in1=ucols, op=ALU.subtract)
                V.tensor_copy(AUG[:, 9 * k + k : 9 * k + 9], RK[:, 0:w_])

            # ---------------- denormalize + gate ----------------
            c_ = V.tensor_copy
            HN = pool.tile([8, 9], F32)
            c_(HN[:, 0:8], AUG[:, 8:72:9])
            V.memset(HN[:, 8:9], 1.0)
            mx_sc, my_sc = SCC[:, 0:1], SCC[:, 1:2]
            s_sc, g_sc = SCC[:, 2:3], SCC[:, 3:4]
            T1 = pool.tile([8, 9], F32)
            H1 = pool.tile([8, 9], F32)
            V.tensor_scalar(out=T1[:, 0:3], in0=HN[:, 0:3], scalar1=s_sc, op0=ALU.mult, scalar2=None)
            V.scalar_tensor_tensor(out=H1[:, 0:3], in0=HN[:, 6:9], scalar=mx_sc,
                                   in1=T1[:, 0:3], op0=ALU.mult, op1=ALU.add)
            V.tensor_scalar(out=T1[:, 3:6], in0=HN[:, 3:6], scalar1=s_sc, op0=ALU.mult, scalar2=None)
            V.scalar_tensor_tensor(out=H1[:, 3:6], in0=HN[:, 6:9], scalar=my_sc,
                                   in1=T1[:, 3:6], op0=ALU.mult, op1=ALU.add)
            c_(H1[:, 6:9], HN[:, 6:9])
            H2 = pool.tile([8, 9], F32)
            H1v = H1[:, :].rearrange("p (r c) -> p r c", r=3)
            H2v = H2[:, :].rearrange("p (r c) -> p r c", r=3)
            V.tensor_scalar(out=H2v[:, :, 0:2], in0=H1v[:, :, 0:2], scalar1=cc.a_ts, op0=ALU.mult, scalar2=None)
            T2 = pool.tile([8, 3], F32)
            T3 = pool.tile([8, 3], F32)
            V.tensor_scalar(out=T2[:, :], in0=H1[:, 0:9:3], scalar1=cc.c_ts, op0=ALU.mult, scalar2=None)
            V.scalar_tensor_tensor(out=T3[:, :], in0=H1[:, 1:9:3], scalar=cc.d_ts,
                                   in1=T2[:, :], op0=ALU.mult, op1=ALU.add)
            V.tensor_tensor(out=H2[:, 2:9:3], in0=T3[:, :], in1=H1[:, 2:9:3], op=ALU.add)
            ABSD = pool.tile([8, 1], F32)
            SGN = pool.tile([8, 1], F32)
            DEN = pool.tile([8, 1], F32)
            RECD = pool.tile([8, 1], F32)
            A.activation(ABSD[:, :], H2[:, 8:9], ACTF.Abs)
            V.tensor_scalar(out=ABSD[:, :], in0=ABSD[:, :], scalar1=1e-8, op0=ALU.max, scalar2=None)
            V.tensor_scalar(out=SGN[:, :], in0=H2[:, 8:9], scalar1=0.0, op0=ALU.is_lt,
                            scalar2=-2.0, op1=ALU.mult)
            V.tensor_scalar(out=SGN[:, :], in0=SGN[:, :], scalar1=1.0, op0=ALU.add, scalar2=None)
            V.tensor_tensor(out=DEN[:, :], in0=ABSD[:, :], in1=SGN[:, :], op=ALU.mult)
            V.reciprocal(RECD[:, :], DEN[:, :])
            V.tensor_scalar(out=H2[:, :], in0=H2[:, :], scalar1=RECD[:, :], op0=ALU.mult, scalar2=None)
            IG = pool.tile([8, 1], F32)
            TI = pool.tile([8, 9], F32)
            OUTt = pool.tile([8, 9], F32)
            V.tensor_scalar(out=IG[:, :], in0=g_sc, scalar1=-1.0, op0=ALU.mult,
                            scalar2=1.0, op1=ALU.add)
            V.tensor_scalar(out=TI[:, :], in0=IEYE_t[:, :], scalar1=IG[:, :], op0=ALU.mult, scalar2=None)
            V.scalar_tensor_tensor(out=OUTt[:, :], in0=H2[:, :], scalar=g_sc,
                                   in1=TI[:, :], op0=ALU.mult, op1=ALU.add)
            S.dma_start(Hout.ap().rearrange("b r c -> b (r c)"), OUTt[:, :])

    nc.compile()
    return nc


# ---------------------------------------------------------------------------
# host wrapper
# ---------------------------------------------------------------------------

_CACHE = {}


def _get(img_h, img_w):
    key = (int(img_h), int(img_w))
    if key not in _CACHE:
        cc = _Consts(*key)
        _CACHE[key] = (cc, _build_program(cc))
    return _CACHE[key]


def _in_maps(cc, flow, mask):
    flow = np.ascontiguousarray(flow, np.float32)
    mask = np.ascontiguousarray(mask, np.float32)
    maps = []
    for c in range(NCORES):
        maps.append({
            "flow": flow[c * BPC : (c + 1) * BPC],
            "mask": mask[c * BPC : (c + 1) * BPC],
            "giF0": cc.giF[0], "giF1": cc.giF[1], "giM": cc.giM,
            "WYF": cc.WYF, "WYM": cc.WYM, "WXF": cc.WXF,
            "C6": cc.C6, "SXYD": cc.SXYD, "G5": cc.G5, "GR5": cc.GR5,
            "IDN": cc.IDN, "EQ": cc.EQ,
        })
    return maps


def run(flow, mask, img_h, img_w, trace=False, **spmd_kwargs):
    cc, nc = _get(img_h, img_w)
    res = bass_utils.run_bass_kernel_spmd(
        nc, _in_maps(cc, flow, mask), list(range(NCORES)), trace=trace, **spmd_kwargs
    )
    out = np.concatenate([res.results[c]["H"] for c in range(NCORES)], axis=0)
    return out.astype(np.float32), res


def kernel(flow, mask, img_h, img_w):
    out, _ = run(flow, mask, img_h, img_w)
    return out



# revision 14
# speedup vs baseline: 1.1313x; 1.1313x over previous
"""Trainium2 Bass kernel for DifferentiableDLT (batched weighted-DLT homography fit).

Contract: kernel(**inputs) takes FULL inputs
    flow (64, 2, 320, 576) f32, mask (64, 1, 320, 576) f32, img_h, img_w
and returns the FULL output (64, 3, 3) f32.

Strategy (pure data parallel, 8 batches/core x 8 cores):
  The 1024 sample points form a fixed separable 32x32 grid.  The needed rows
  follow an affine pattern y0[k] = 16 + 37*(k//4) + 9*(k%4) (k=31 lands on
  rows 302/303 with wy patched to 1.0, exact since the true sample sits on
  row 303).  So the row fetch is 8 static 3D-access-pattern DMAs -- no
  gpsimd gather, no index tables.

  Per core:
    1. 4 flow DMAs (one per k%4 slot): [bc 16][k4 8][pair-row 1152] and 4
       mask DMAs [b 8][k4 8][1152] issued from sync/scalar HWDGE queues at
       t=0, plus one packed constants blob.
    2. x-lerp directly on strided column views (9 uniform-stride runs),
       then y-lerp -> sampF [128=(k4,b,c)][(s,i)=128], sampM [64=(k4,b)].
    3. One PE matmul fuses transpose + image scaling + grid offset + a
       compile-time Hartley normalization (constant T from the source grid;
       dst stats differ from it only by O(flow/image) ~ 1e-4 relative, and
       the solve's eps-regularization sensitivity to T is ~1e-8).
    4. D = [w, w*p, w*q, w*(p^2+q^2)] (raw normalized coords, no centering);
       moments = C9^T @ D in ONE PE matmul (stationary C9 [128,72] holds 9
       redundant features x 8 point-tiles); tile-diagonal extracted with 8
       copies + reduce.
    5. Transpose moments to [batch, moment]; assemble the 8x9 augmented
       normal equations directly with ~12 strided copies (no EQ matmul);
       unpivoted Gauss-Jordan; denormalize with immediate constants;
       sign/scale fix; support gate; DMA out (8,3,3).
"""

import dataclasses
import math
import numpy as np

import concourse.bass as bass
import concourse.bacc as bacc
import concourse.mybir as mybir
from concourse import tile
from concourse import bass_utils

F32 = mybir.dt.float32
ALU = mybir.AluOpType

NCORES = 8
BPC = 8          # batches per core
HF, WF = 320, 576
NG = 32          # grid is NG x NG points
NPTS = NG * NG
EPS = 1e-6

# constant-blob column layout
C_SXY = 0        # [128, 128] diag transpose-scale
C_C9 = 128       # [128, 72] point features (9F x 8 tiles)
C_WY4 = 200      # [128, 4] flow y-weights
C_G6 = 204       # [6, 128] grid-offset stationary
C_GR6 = 332      # [6, 128] grid-offset moving
C_IDN = 460      # [64, 64] identity
C_WX = 524       # [128, 32] x-weights
C_WY4M = 556     # [64, 4] mask y-weights
C_NCOL = 560


def _grid_1d(size, n):
    m = int(size * 0.05)
    return np.linspace(m, size - m - 1, n, dtype=np.float32)


def _segments(x0):
    segs = []
    i = 0
    n = len(x0)
    while i < n:
        if i == n - 1:
            segs.append((i, 1, 1))
            break
        st = x0[i + 1] - x0[i]
        j = i + 1
        while j + 1 < n and x0[j + 1] - x0[j] == st:
            j += 1
        segs.append((i, j - i + 1, int(st)))
        i = j + 1
    return segs


class _Consts:
    def __init__(self, img_h, img_w):
        ys = _grid_1d(HF, NG)
        xs = _grid_1d(WF, NG)
        k = np.arange(NG)
        y0 = 16 + 37 * (k // 4) + 9 * (k % 4)
        x0 = np.floor(xs).astype(np.int64)
        wy = (ys.astype(np.float64) - y0)
        wy[31] = 1.0  # rows (302,303) loaded; true sample is row 303 exactly
        wx = (xs.astype(np.float64) - x0)
        self.segs = _segments(x0)
        self.x0 = x0
        sx = float(np.float32((img_w - 1) / max(WF - 1, 1)))
        sy = float(np.float32((img_h - 1) / max(HF - 1, 1)))

        # compile-time Hartley T from the source grid (used for src AND dst)
        gx = xs.astype(np.float64)[np.arange(NPTS) % NG]
        gy = ys.astype(np.float64)[np.arange(NPTS) // NG]
        sxi, syi = gx * sx, gy * sy
        mx, my = sxi.mean(), syi.mean()
        s_c = max(np.sqrt((sxi - mx) ** 2 + (syi - my) ** 2).mean()
                  / math.sqrt(2.0), 1e-8)
        a_t = 1.0 / s_c
        self.s_c, self.mx, self.my, self.a_t = s_c, mx, my, a_t

        u = (xs.astype(np.float64) * sx - mx) * a_t   # per i
        v = (ys.astype(np.float64) * sy - my) * a_t   # per k

        CB = np.zeros((128, C_NCOL), np.float64)
        # SXYDn: diag, n = (b*2+c)*8 + k4 -> scale = (sx|sy) * a_t
        n = np.arange(128)
        c_n = (n // 8) % 2
        k4_n = n % 8
        CB[:, C_SXY:C_SXY + 128] = np.eye(128) * np.where(c_n == 0, sx, sy) * a_t
        # C9[m=(s,i), t*9+f], F = (uu, uv, u, uv, vv, v, u, v, 1)
        m = np.arange(128)
        s_m, i_m = m // 32, m % 32
        for t in range(8):
            U = u[i_m]
            V = v[t * 4 + s_m]
            F9 = np.stack([U * U, U * V, U, U * V, V * V, V, U, V,
                           np.ones_like(U)], -1)  # (128, 9)
            CB[:, C_C9 + 9 * t:C_C9 + 9 * t + 9] = F9
        # WY4[p, s] = wy[(p%8)*4 + s]  (partition p = bc*8 + k4)
        CB[:, C_WY4:C_WY4 + 4] = wy[(np.arange(128) % 8)[:, None] * 4
                                    + np.arange(4)[None, :]]
        # G6/GR6: psF[m, n] += sum_r G6[r, m] * GR6[r, n]
        G6 = np.zeros((6, 128))
        GR6 = np.zeros((6, 128))
        for sp in range(4):
            G6[sp] = (s_m == sp)
            GR6[sp] = np.where(
                c_n == 1,
                (ys.astype(np.float64)[4 * k4_n + sp] * sy - my) * a_t, 0.0)
        G6[4] = xs.astype(np.float64)[i_m]
        GR6[4] = np.where(c_n == 0, sx * a_t, 0.0)
        G6[5] = 1.0
        GR6[5] = np.where(c_n == 0, -mx * a_t, 0.0)
        CB[0:6, C_G6:C_G6 + 128] = G6
        CB[0:6, C_GR6:C_GR6 + 128] = GR6
        CB[0:64, C_IDN:C_IDN + 64] = np.eye(64)
        CB[:, C_WX:C_WX + 32] = np.tile(wx[None, :], (128, 1))
        # WY4M[p, s] = wy[(p%8)*4 + s] for mask partitions p = b*8 + k4
        CB[0:64, C_WY4M:C_WY4M + 4] = wy[(np.arange(64) % 8)[:, None] * 4
                                         + np.arange(4)[None, :]]
        self.CB = CB.astype(np.float32)


def _flat(ap):
    return ap.rearrange("b c h w -> (b c h w)").unsqueeze(0)


def _build_program(cc: _Consts):
    nc = bacc.Bacc("TRN2", target_bir_lowering=False, debug=False,
                   num_swdge_queues=2)

    flow = nc.dram_tensor("flow", [BPC, 2, HF, WF], F32, kind="ExternalInput")
    mask = nc.dram_tensor("mask", [BPC, 1, HF, WF], F32, kind="ExternalInput")
    CBd = nc.dram_tensor("CB", [128, C_NCOL], F32, kind="ExternalInput")
    Hout = nc.dram_tensor("H", [BPC, 3, 3], F32, kind="ExternalOutput")

    V = nc.vector
    A = nc.scalar
    T = nc.tensor
    S = nc.sync

    with tile.TileContext(nc) as tc:
        with (
            tc.tile_pool(name="sb", bufs=1) as pool,
            tc.tile_pool(name="ps", bufs=1, space="PSUM") as psp,
        ):
            # ---------------- DMAs in ----------------
            CB_t = pool.tile([128, C_NCOL], F32, tag="CB")
            A.dma_start(CB_t[:, :], CBd[:])

            tF = pool.tile([128, 4, 1152], F32)   # [p=(b,c,k4)][s][pair row]
            tM = pool.tile([64, 4, 1152], F32)    # [p=(b,k4)][s][pair row]
            srcF = _flat(flow.ap())
            srcM = _flat(mask.ap())
            for s in range(4):
                f3 = dataclasses.replace(
                    srcF, ap=[[HF * WF, 16], [37 * WF, 8], [1, 1152]],
                    offset=(16 + 9 * s) * WF)
                S.dma_start(tF[:, s, :], f3)
            for s in range(4):
                m3 = dataclasses.replace(
                    srcM, ap=[[HF * WF, 8], [37 * WF, 8], [1, 1152]],
                    offset=(16 + 9 * s) * WF)
                A.dma_start(tM[:, s, :], m3)

            SXYDn = CB_t[:, C_SXY:C_SXY + 128]
            C9 = CB_t[:, C_C9:C_C9 + 72]
            WY4 = CB_t[:, C_WY4:C_WY4 + 4]
            G6 = CB_t[0:6, C_G6:C_G6 + 128]
            GR6 = CB_t[0:6, C_GR6:C_GR6 + 128]
            IDN = CB_t[0:64, C_IDN:C_IDN + 64]
            WXT = CB_t[:, C_WX:C_WX + 32]
            WY4M = CB_t[0:64, C_WY4M:C_WY4M + 4]

            IEYE = pool.tile([8, 9], F32, tag="IEYE")
            V.memset(IEYE[:, :], 0.0)
            V.memset(IEYE[:, 0:9:4], 1.0)

            # ---------------- flow interp ----------------
            # x-lerp on strided views, then y-lerp
            tFv = tF[:, :, :].rearrange("p s (a w) -> p s a w", a=2)
            XD = pool.tile([128, 4, 2, 32], F32)   # scratch (g1-g0)*wx
            XL = pool.tile([128, 4, 2, 32], F32)   # x-lerped
            for (i0, L, st) in cc.segs:
                b0 = int(cc.x0[i0])
                hi = b0 + (L - 1) * st + 1
                g0 = tFv[:, :, :, b0:hi:st]
                g1 = tFv[:, :, :, b0 + 1:hi + 1:st]
                d = XD[:, :, :, i0:i0 + L]
                wxv = WXT[:, i0:i0 + L].unsqueeze(1).unsqueeze(1) \
                    .broadcast_to([128, 4, 2, L])
                V.tensor_tensor(out=d, in0=g1, in1=g0, op=ALU.subtract)
                V.tensor_tensor(out=d, in0=d, in1=wxv, op=ALU.mult)
                V.tensor_tensor(out=XL[:, :, :, i0:i0 + L], in0=d, in1=g0,
                                op=ALU.add)
            sampF = pool.tile([128, 128], F32)     # [(k4,b,c)][(s,i)]
            YD = pool.tile([128, 128], F32)
            wyv = WY4[:, :].unsqueeze(2).broadcast_to([128, 4, 32])
            sF = sampF[:, :].rearrange("p (s i) -> p s i", s=4)
            yD = YD[:, :].rearrange("p (s i) -> p s i", s=4)
            V.tensor_tensor(out=yD, in0=XL[:, :, 1, :], in1=XL[:, :, 0, :],
                            op=ALU.subtract)
            V.tensor_tensor(out=yD, in0=yD, in1=wyv, op=ALU.mult)
            V.tensor_tensor(out=sF, in0=yD, in1=XL[:, :, 0, :], op=ALU.add)

            # ---------------- mask interp ----------------
            tMv = tM[:, :, :].rearrange("p s (a w) -> p s a w", a=2)
            XDM = pool.tile([64, 4, 2, 32], F32)
            XLM = pool.tile([64, 4, 2, 32], F32)
            for (i0, L, st) in cc.segs:
                b0 = int(cc.x0[i0])
                hi = b0 + (L - 1) * st + 1
                g0 = tMv[:, :, :, b0:hi:st]
                g1 = tMv[:, :, :, b0 + 1:hi + 1:st]
                d = XDM[:, :, :, i0:i0 + L]
                wxv = WXT[0:64, i0:i0 + L].unsqueeze(1).unsqueeze(1) \
                    .broadcast_to([64, 4, 2, L])
                V.tensor_tensor(out=d, in0=g1, in1=g0, op=ALU.subtract)
                V.tensor_tensor(out=d, in0=d, in1=wxv, op=ALU.mult)
                V.tensor_tensor(out=XLM[:, :, :, i0:i0 + L], in0=d, in1=g0,
                                op=ALU.add)
            sampM = pool.tile([64, 128], F32)      # [(k4,b)][(s,i)]
            YDM = pool.tile([64, 128], F32)
            wyvm = WY4M[:, :].unsqueeze(2).broadcast_to([64, 4, 32])
            sM = sampM[:, :].rearrange("p (s i) -> p s i", s=4)
            yDM = YDM[:, :].rearrange("p (s i) -> p s i", s=4)
            V.tensor_tensor(out=yDM, in0=XLM[:, :, 1, :], in1=XLM[:, :, 0, :],
                            op=ALU.subtract)
            V.tensor_tensor(out=yDM, in0=yDM, in1=wyvm, op=ALU.mult)
            V.tensor_tensor(out=sM, in0=yDM, in1=XLM[:, :, 0, :], op=ALU.add)

            # ------- transpose + scale + grid + normalize (one PE pass) -----
            psF = psp.tile([128, 128], F32)
            T.matmul(psF[:, :], sampF[:, :], SXYDn, start=True, stop=False)
            T.matmul(psF[:, :], G6, GR6, start=False, stop=True)
            PQs = pool.tile([128, 128], F32)  # normalized dst [m=(s,i)][(b,c,t)]
            V.tensor_copy(PQs[:, :], psF[:, :])
            psM = psp.tile([128, 64], F32)
            T.transpose(psM[:, :], sampM[:, :], IDN)
            SM = pool.tile([128, 64], F32)    # mask sample [m][(b,t)]
            V.tensor_copy(SM[:, :], psM[:, :])

            # ---------------- D = [w, w*p, w*q, w*r2] ----------------
            D = pool.tile([128, 256], F32)    # [m][(t,q,b)]
            Dv = D[:, :].rearrange("p (t q b) -> p q t b", q=4, b=8)
            V.tensor_scalar(out=Dv[:, 0, :, :],
                            in0=SM[:, :].rearrange("p (b t) -> p t b", t=8),
                            scalar1=0.0, op0=ALU.max, scalar2=None)
            d12 = D[:, :].rearrange("p (t q b) -> p t q b", q=4, b=8)[:, :, 1:3, :]
            pq12 = PQs[:, :].rearrange("p (b c t) -> p t c b", t=8, b=8, c=2)
            wb2 = Dv[:, 0, :, :].unsqueeze(2).broadcast_to([128, 8, 2, 8])
            V.tensor_tensor(out=d12, in0=pq12, in1=wb2, op=ALU.mult)
            SQ = pool.tile([128, 128], F32)
            V.tensor_tensor(out=SQ[:, :], in0=PQs[:, :], in1=PQs[:, :],
                            op=ALU.mult)
            R2 = pool.tile([128, 64], F32)
            sq3 = SQ[:, :].rearrange("p (b c t) -> p c t b", t=8, b=8, c=2)
            V.tensor_tensor(out=R2[:, :].rearrange("p (t b) -> p t b", t=8),
                            in0=sq3[:, 0, :, :], in1=sq3[:, 1, :, :], op=ALU.add)
            V.tensor_tensor(out=Dv[:, 3, :, :],
                            in0=R2[:, :].rearrange("p (t b) -> p t b", t=8),
                            in1=Dv[:, 0, :, :], op=ALU.mult)

            # ------- moments: psMom[f, (q,b)] = sum_t C9_t^T D_t ----------
            psMom = psp.tile([9, 32], F32)
            for t in range(8):
                T.matmul(psMom[:, :], C9[:, 9 * t:9 * t + 9],
                         D[:, 32 * t:32 * t + 32], start=(t == 0),
                         stop=(t == 7))
            Msb = pool.tile([9, 32], F32)
            V.tensor_copy(Msb[:, :], psMom[:, :])
            # per-q PE transposes: [8 = batch, 9 = feature] tiles
            MQT = [pool.tile([8, 9], F32, name=f"MQT{q}") for q in range(4)]
            psQ = [psp.tile([8, 9], F32, name=f"psQ{q}") for q in range(4)]
            for q in range(4):
                T.transpose(psQ[q][:, :], Msb[:, 8 * q:8 * q + 8],
                            IDN[0:9, 0:9])
                V.tensor_copy(MQT[q][:, :], psQ[q][:, :])
            M0, Mp, Mq, Mr = MQT

            # ---------------- assemble AUG [8, 72] ----------------
            AUG = pool.tile([8, 72], F32)
            V.memset(AUG[:, :], 0.0)
            a33 = AUG[:, :].rearrange("p (r c) -> p r c", r=8)

            def v33(mt, tr=False):
                vv = mt[:, :].rearrange("p (r c) -> p r c", r=3)
                return vv.rearrange("p r c -> p c r") if tr else vv

            V.tensor_copy(a33[:, 0:3, 0:3], v33(M0))
            V.tensor_copy(a33[:, 3:6, 3:6], v33(M0))
            V.tensor_scalar(out=a33[:, 0:3, 6:8], in0=v33(Mp)[:, :, 0:2],
                            scalar1=-1.0, op0=ALU.mult, scalar2=None)
            V.tensor_scalar(out=a33[:, 3:6, 6:8], in0=v33(Mq)[:, :, 0:2],
                            scalar1=-1.0, op0=ALU.mult, scalar2=None)
            V.tensor_scalar(out=a33[:, 6:8, 0:3], in0=v33(Mp, tr=True)[:, 0:2, :],
                            scalar1=-1.0, op0=ALU.mult, scalar2=None)
            V.tensor_scalar(out=a33[:, 6:8, 3:6], in0=v33(Mq, tr=True)[:, 0:2, :],
                            scalar1=-1.0, op0=ALU.mult, scalar2=None)
            V.tensor_copy(a33[:, 6:8, 6:8], v33(Mr)[:, 0:2, 0:2])
            V.tensor_copy(AUG[:, 8:27:9], Mp[:, 6:9])
            V.tensor_copy(AUG[:, 35:54:9], Mq[:, 6:9])
            V.tensor_scalar(out=AUG[:, 62:72:9], in0=Mr[:, 6:8],
                            scalar1=-1.0, op0=ALU.mult, scalar2=None)
            V.tensor_scalar(out=AUG[:, 0:71:10], in0=AUG[:, 0:71:10],
                            scalar1=EPS, op0=ALU.add, scalar2=None)
            # support gate from AUG[2,2] = sum(w) + eps
            GT = pool.tile([8, 1], F32)
            V.tensor_scalar(out=GT[:, :], in0=AUG[:, 20:21],
                            scalar1=NPTS * 1e-4 + EPS, op0=ALU.is_gt,
                            scalar2=None)

            # ---------------- Gauss-Jordan ----------------
            RK = pool.tile([8, 9], F32)
            PIV = pool.tile([8, 1], F32)
            U8 = pool.tile([8, 72], F32)
            for kk in range(8):
                w_ = 9 - kk
                V.reciprocal(PIV[:, :], AUG[:, 9 * kk + kk:9 * kk + kk + 1])
                V.tensor_scalar(out=RK[:, 0:w_], in0=AUG[:, 9 * kk + kk:9 * kk + 9],
                                scalar1=PIV[:, :], op0=ALU.mult, scalar2=None)
                fcol = AUG[:, kk:72:9].unsqueeze(2).broadcast_to([8, 8, w_])
                rkb = RK[:, 0:w_].unsqueeze(1).broadcast_to([8, 8, w_])
                ucols = U8[:, :].rearrange("p (r c) -> p r c", r=8)[:, :, 0:w_]
                acols = AUG[:, :].rearrange("p (r c) -> p r c", r=8)[:, :, kk:9]
                V.tensor_tensor(out=ucols, in0=fcol, in1=rkb, op=ALU.mult)
                V.tensor_tensor(out=acols, in0=acols, in1=ucols, op=ALU.subtract)
                V.tensor_copy(AUG[:, 9 * kk + kk:9 * kk + 9], RK[:, 0:w_])

            # ---------------- denormalize (immediate T) ----------------
            c_ = V.tensor_copy
            HN = pool.tile([8, 9], F32)
            c_(HN[:, 0:8], AUG[:, 8:72:9])
            V.memset(HN[:, 8:9], 1.0)
            s_c, mx, my, a_t = cc.s_c, cc.mx, cc.my, cc.a_t
            T1 = pool.tile([8, 9], F32)
            H1 = pool.tile([8, 9], F32)
            V.tensor_scalar(out=T1[:, 0:3], in0=HN[:, 0:3], scalar1=s_c,
                            op0=ALU.mult, scalar2=None)
            V.scalar_tensor_tensor(out=H1[:, 0:3], in0=HN[:, 6:9], scalar=mx,
                                   in1=T1[:, 0:3], op0=ALU.mult, op1=ALU.add)
            V.tensor_scalar(out=T1[:, 3:6], in0=HN[:, 3:6], scalar1=s_c,
                            op0=ALU.mult, scalar2=None)
            V.scalar_tensor_tensor(out=H1[:, 3:6], in0=HN[:, 6:9], scalar=my,
                                   in1=T1[:, 3:6], op0=ALU.mult, op1=ALU.add)
            c_(H1[:, 6:9], HN[:, 6:9])
            H2 = pool.tile([8, 9], F32)
            H1v = H1[:, :].rearrange("p (r c) -> p r c", r=3)
            H2v = H2[:, :].rearrange("p (r c) -> p r c", r=3)
            V.tensor_scalar(out=H2v[:, :, 0:2], in0=H1v[:, :, 0:2],
                            scalar1=a_t, op0=ALU.mult, scalar2=None)
            T2 = pool.tile([8, 3], F32)
            T3 = pool.tile([8, 3], F32)
            V.tensor_scalar(out=T2[:, :], in0=H1[:, 0:9:3], scalar1=-mx * a_t,
                            op0=ALU.mult, scalar2=None)
            V.scalar_tensor_tensor(out=T3[:, :], in0=H1[:, 1:9:3],
                                   scalar=-my * a_t, in1=T2[:, :],
                                   op0=ALU.mult, op1=ALU.add)
            V.tensor_tensor(out=H2[:, 2:9:3], in0=T3[:, :], in1=H1[:, 2:9:3],
                            op=ALU.add)
            # sign/scale fix: H /= sign(H22)*max(|H22|, 1e-8)
            NEG = pool.tile([8, 1], F32)
            ABSD = pool.tile([8, 1], F32)
            SGN = pool.tile([8, 1], F32)
            DEN = pool.tile([8, 1], F32)
            RECD = pool.tile([8, 1], F32)
            V.tensor_scalar(out=NEG[:, :], in0=H2[:, 8:9], scalar1=-1.0,
                            op0=ALU.mult, scalar2=None)
            V.tensor_tensor(out=ABSD[:, :], in0=NEG[:, :], in1=H2[:, 8:9],
                            op=ALU.max)
            V.tensor_scalar(out=ABSD[:, :], in0=ABSD[:, :], scalar1=1e-8,
                            op0=ALU.max, scalar2=None)
            V.tensor_scalar(out=SGN[:, :], in0=H2[:, 8:9], scalar1=0.0,
                            op0=ALU.is_lt, scalar2=-2.0, op1=ALU.mult)
            V.tensor_scalar(out=SGN[:, :], in0=SGN[:, :], scalar1=1.0,
                            op0=ALU.add, scalar2=None)
            V.tensor_tensor(out=DEN[:, :], in0=ABSD[:, :], in1=SGN[:, :],
                            op=ALU.mult)
            V.reciprocal(RECD[:, :], DEN[:, :])
            V.tensor_scalar(out=H2[:, :], in0=H2[:, :], scalar1=RECD[:, :],
                            op0=ALU.mult, scalar2=None)
            # support gate
            IG = pool.tile([8, 1], F32)
            TI = pool.tile([8, 9], F32)
            OUTt = pool.tile([8, 9], F32)
            V.tensor_scalar(out=IG[:, :], in0=GT[:, :], scalar1=-1.0,
                            op0=ALU.mult, scalar2=1.0, op1=ALU.add)
            V.tensor_scalar(out=TI[:, :], in0=IEYE[:, :], scalar1=IG[:, :],
                            op0=ALU.mult, scalar2=None)
            V.scalar_tensor_tensor(out=OUTt[:, :], in0=H2[:, :], scalar=GT[:, :],
                                   in1=TI[:, :], op0=ALU.mult, op1=ALU.add)
            S.dma_start(Hout.ap().rearrange("b r c -> b (r c)"), OUTt[:, :])

    nc.compile()
    return nc


# ---------------------------------------------------------------------------
# host wrapper
# ---------------------------------------------------------------------------

_CACHE = {}


def _get(img_h, img_w):
    key = (int(img_h), int(img_w))
    if key not in _CACHE:
        cc = _Consts(*key)
        _CACHE[key] = (cc, _build_program(cc))
    return _CACHE[key]


def _in_maps(cc, flow, mask):
    flow = np.ascontiguousarray(flow, np.float32)
    mask = np.ascontiguousarray(mask, np.float32)
    return [{
        "flow": flow[c * BPC:(c + 1) * BPC],
        "mask": mask[c * BPC:(c + 1) * BPC],
        "CB": cc.CB,
    } for c in range(NCORES)]


def run(flow, mask, img_h, img_w, trace=False, **spmd_kwargs):
    cc, nc = _get(img_h, img_w)
    res = bass_utils.run_bass_kernel_spmd(
        nc, _in_maps(cc, flow, mask), list(range(NCORES)), trace=trace,
        **spmd_kwargs)
    out = np.concatenate([res.results[c]["H"] for c in range(NCORES)], axis=0)
    return out.astype(np.float32), res


def kernel(flow, mask, img_h, img_w):
    out, _ = run(flow, mask, img_h, img_w)
    return out


# revision 22
# speedup vs baseline: 1.2073x; 1.0671x over previous
"""Trainium2 Bass kernel for DifferentiableDLT (batched weighted-DLT homography fit).

Contract: kernel(**inputs) takes FULL inputs
    flow (64, 2, 320, 576) f32, mask (64, 1, 320, 576) f32, img_h, img_w
and returns the FULL output (64, 3, 3) f32.

Strategy (pure data parallel, 8 batches/core x 8 cores):
  The 1024 sample points form a fixed separable 32x32 grid.  The needed rows
  follow an affine pattern y0[k] = 16 + 37*(k//4) + 9*(k%4) (k=31 lands on
  rows 302/303 with wy patched to 1.0, exact since the true sample sits on
  row 303).  So the row fetch is 8 static 3D-access-pattern DMAs -- no
  gpsimd gather, no index tables.

  Per core:
    1. 4 flow DMAs (one per k%4 slot): [bc 16][k4 8][pair-row 1152] and 4
       mask DMAs [b 8][k4 8][1152] issued from sync/scalar HWDGE queues at
       t=0, plus one packed constants blob.
    2. x-lerp directly on strided column views (9 uniform-stride runs),
       then y-lerp -> sampF [128=(k4,b,c)][(s,i)=128], sampM [64=(k4,b)].
    3. One PE matmul fuses transpose + image scaling + grid offset + a
       compile-time Hartley normalization (constant T from the source grid;
       dst stats differ from it only by O(flow/image) ~ 1e-4 relative, and
       the solve's eps-regularization sensitivity to T is ~1e-8).
    4. D = [w, w*p, w*q, w*(p^2+q^2)] (raw normalized coords, no centering);
       moments = C9^T @ D in ONE PE matmul (stationary C9 [128,72] holds 9
       redundant features x 8 point-tiles); tile-diagonal extracted with 8
       copies + reduce.
    5. Transpose moments to [batch, moment]; assemble the 8x9 augmented
       normal equations directly with ~12 strided copies (no EQ matmul);
       unpivoted Gauss-Jordan; denormalize with immediate constants;
       sign/scale fix; support gate; DMA out (8,3,3).
"""

import dataclasses
import math
import numpy as np

import concourse.bass as bass
import concourse.bacc as bacc
import concourse.mybir as mybir
from concourse import tile
from concourse import bass_utils

F32 = mybir.dt.float32
ALU = mybir.AluOpType

NCORES = 8
BPC = 8          # batches per core
HF, WF = 320, 576
NG = 32          # grid is NG x NG points
NPTS = NG * NG
EPS = 1e-6

# constant-blob column layout
C_SXY = 0        # [128, 128] diag transpose-scale
C_C9 = 128       # [128, 72] point features (9F x 8 tiles)
C_WY4 = 200      # [128, 4] flow y-weights
C_G6 = 204       # [6, 128] grid-offset stationary
C_GR6 = 332      # [6, 128] grid-offset moving
C_IDN = 460      # [64, 64] identity
C_WX = 524       # [128, 32] x-weights
C_WY4M = 556     # [64, 4] mask y-weights
C_NCOL = 560


def _grid_1d(size, n):
    m = int(size * 0.05)
    return np.linspace(m, size - m - 1, n, dtype=np.float32)


def _segments(x0):
    segs = []
    i = 0
    n = len(x0)
    while i < n:
        if i == n - 1:
            segs.append((i, 1, 1))
            break
        st = x0[i + 1] - x0[i]
        j = i + 1
        while j + 1 < n and x0[j + 1] - x0[j] == st:
            j += 1
        segs.append((i, j - i + 1, int(st)))
        i = j + 1
    return segs


class _Consts:
    def __init__(self, img_h, img_w):
        ys = _grid_1d(HF, NG)
        xs = _grid_1d(WF, NG)
        k = np.arange(NG)
        y0 = 16 + 37 * (k // 4) + 9 * (k % 4)
        x0 = np.floor(xs).astype(np.int64)
        wy = (ys.astype(np.float64) - y0)
        wy[31] = 1.0  # rows (302,303) loaded; true sample is row 303 exactly
        wx = (xs.astype(np.float64) - x0)
        # x0 structure: i=0..3 -> 28+[0,16,33,50]; i=4a'+r+4 -> 94+67a'+17r
        assert x0[0] == 28 and x0[1] == 44 and x0[2] == 61 and x0[3] == 78
        assert all(x0[4 + 4 * a + r] == 94 + 67 * a + 17 * r
                   for a in range(7) for r in range(4))
        self.x0 = x0
        sx = float(np.float32((img_w - 1) / max(WF - 1, 1)))
        sy = float(np.float32((img_h - 1) / max(HF - 1, 1)))

        # compile-time Hartley T from the source grid (used for src AND dst)
        gx = xs.astype(np.float64)[np.arange(NPTS) % NG]
        gy = ys.astype(np.float64)[np.arange(NPTS) // NG]
        sxi, syi = gx * sx, gy * sy
        mx, my = sxi.mean(), syi.mean()
        s_c = max(np.sqrt((sxi - mx) ** 2 + (syi - my) ** 2).mean()
                  / math.sqrt(2.0), 1e-8)
        a_t = 1.0 / s_c
        self.s_c, self.mx, self.my, self.a_t = s_c, mx, my, a_t

        u = (xs.astype(np.float64) * sx - mx) * a_t   # per i
        v = (ys.astype(np.float64) * sy - my) * a_t   # per k

        CB = np.zeros((128, C_NCOL), np.float64)
        # SXYDn: diag, n = (b*2+c)*8 + k4 -> scale = (sx|sy) * a_t
        n = np.arange(128)
        c_n = (n // 8) % 2
        k4_n = n % 8
        CB[:, C_SXY:C_SXY + 128] = np.eye(128) * np.where(c_n == 0, sx, sy) * a_t
        # C9[m=(s,i), t*9+f], F = (uu, uv, u, uv, vv, v, u, v, 1)
        m = np.arange(128)
        s_m, i_m = m // 32, m % 32
        for t in range(8):
            U = u[i_m]
            V = v[t * 4 + s_m]
            F9 = np.stack([U * U, U * V, U, U * V, V * V, V, U, V,
                           np.ones_like(U)], -1)  # (128, 9)
            CB[:, C_C9 + 9 * t:C_C9 + 9 * t + 9] = F9
        # WY4[p, s] = wy[(p%8)*4 + s]  (partition p = bc*8 + k4)
        CB[:, C_WY4:C_WY4 + 4] = wy[(np.arange(128) % 8)[:, None] * 4
                                    + np.arange(4)[None, :]]
        # G6/GR6: psF[m, n] += sum_r G6[r, m] * GR6[r, n]
        G6 = np.zeros((6, 128))
        GR6 = np.zeros((6, 128))
        for sp in range(4):
            G6[sp] = (s_m == sp)
            GR6[sp] = np.where(
                c_n == 1,
                (ys.astype(np.float64)[4 * k4_n + sp] * sy - my) * a_t, 0.0)
        G6[4] = xs.astype(np.float64)[i_m]
        GR6[4] = np.where(c_n == 0, sx * a_t, 0.0)
        G6[5] = 1.0
        GR6[5] = np.where(c_n == 0, -mx * a_t, 0.0)
        CB[0:6, C_G6:C_G6 + 128] = G6
        CB[0:6, C_GR6:C_GR6 + 128] = GR6
        CB[0:64, C_IDN:C_IDN + 64] = np.eye(64)
        CB[:, C_WX:C_WX + 32] = np.tile(wx[None, :], (128, 1))
        # WY4M[p, s] = wy[(p%8)*4 + s] for mask partitions p = b*8 + k4
        CB[0:64, C_WY4M:C_WY4M + 4] = wy[(np.arange(64) % 8)[:, None] * 4
                                         + np.arange(4)[None, :]]
        self.CB = CB.astype(np.float32)


def _flat(ap):
    return ap.rearrange("b c h w -> (b c h w)").unsqueeze(0)


def _build_program(cc: _Consts):
    nc = bacc.Bacc("TRN2", target_bir_lowering=False, debug=False,
                   num_swdge_queues=2)

    flow = nc.dram_tensor("flow", [BPC, 2, HF, WF], F32, kind="ExternalInput")
    mask = nc.dram_tensor("mask", [BPC, 1, HF, WF], F32, kind="ExternalInput")
    CBd = nc.dram_tensor("CB", [128, C_NCOL], F32, kind="ExternalInput")
    Hout = nc.dram_tensor("H", [BPC, 3, 3], F32, kind="ExternalOutput")

    V = nc.vector
    A = nc.scalar
    T = nc.tensor
    S = nc.sync

    with tile.TileContext(nc) as tc:
        with (
            tc.tile_pool(name="sb", bufs=1) as pool,
            tc.tile_pool(name="ps", bufs=1, space="PSUM") as psp,
        ):
            # ---------------- DMAs in ----------------
            CB_t = pool.tile([128, C_NCOL], F32, tag="CB")
            A.dma_start(CB_t[:, :], CBd[:])

            tF = pool.tile([128, 4, 1152], F32)   # [p=(b,c,k4)][s][pair row]
            tM = pool.tile([64, 4, 1152], F32)    # [p=(b,k4)][s][pair row]
            srcF = _flat(flow.ap())
            srcM = _flat(mask.ap())
            # flow slots s0/s1 on sync queue, s2/s3 on scalar (after CB);
            # mask on the gpsimd software-DGE queue (3rd parallel channel)
            for s in range(4):
                f3 = dataclasses.replace(
                    srcF, ap=[[HF * WF, 16], [37 * WF, 8], [1, 1152]],
                    offset=(16 + 9 * s) * WF)
                (S if s < 2 else A).dma_start(tF[:, s, :], f3)
            for s in range(4):
                m3 = dataclasses.replace(
                    srcM, ap=[[HF * WF, 8], [37 * WF, 8], [1, 1152]],
                    offset=(16 + 9 * s) * WF)
                nc.gpsimd.dma_start(tM[:, s, :], m3)

            SXYDn = CB_t[:, C_SXY:C_SXY + 128]
            C9 = CB_t[:, C_C9:C_C9 + 72]
            WY4 = CB_t[:, C_WY4:C_WY4 + 4]
            G6 = CB_t[0:6, C_G6:C_G6 + 128]
            GR6 = CB_t[0:6, C_GR6:C_GR6 + 128]
            IDN = CB_t[0:64, C_IDN:C_IDN + 64]
            WXT = CB_t[:, C_WX:C_WX + 32]
            WY4M = CB_t[0:64, C_WY4M:C_WY4M + 4]

            IEYE = pool.tile([8, 9], F32, tag="IEYE")
            V.memset(IEYE[:, :], 0.0)
            V.memset(IEYE[:, 0:9:4], 1.0)

            # ---------------- interp (affine-x views) ----------------
            # x0 families: i=0,1 (base 28, step 16); i=2,3 (base 61, step 17);
            # i=4+4a'+r (base 94, strides 67/17).  x-lerp directly on strided
            # views of the raw pair rows (s,a merged), then y-lerp.
            def xy_interp(tile_t, np_, WXv, WYv, samp, XD, XL, s_lo, s_hi):
                ns = s_hi - s_lo
                nsa = 2 * ns
                flat = tile_t[:, :, :].rearrange("p s e -> p (s e)")
                xlf = XL[:, :, :].rearrange("p sa i -> p (sa i)")
                xdf = XD[:, :, :].rearrange("p sa i -> p (sa i)")
                for (fam_off, fam_i0, dims, odims) in (
                    (28, 0, [[16, 2]], [[1, 2]]),
                    (61, 2, [[17, 2]], [[1, 2]]),
                    (94, 4, [[67, 7], [17, 4]], [[4, 7], [1, 4]]),
                ):
                    src_ap = [list(flat.ap[0]), [576, nsa]] + \
                        [list(x) for x in dims]
                    g0 = dataclasses.replace(
                        flat, ap=[list(x) for x in src_ap],
                        offset=flat.offset + s_lo * 1152 + fam_off)
                    g1 = dataclasses.replace(
                        flat, ap=[list(x) for x in src_ap],
                        offset=flat.offset + s_lo * 1152 + fam_off + 1)
                    out_ap = [list(xlf.ap[0]), [32, nsa]] + \
                        [list(x) for x in odims]
                    d_o = dataclasses.replace(
                        xlf, ap=[list(x) for x in out_ap],
                        offset=xlf.offset + fam_i0)
                    d_t = dataclasses.replace(
                        xdf, ap=[list(x) for x in out_ap],
                        offset=xdf.offset + fam_i0)
                    if len(dims) == 2:
                        wxv = WXv[:, 4:32].rearrange("p (a r) -> p a r", a=7) \
                            .unsqueeze(1).broadcast_to([np_, nsa, 7, 4])
                    else:
                        wxv = WXv[:, fam_i0:fam_i0 + 2].unsqueeze(1) \
                            .broadcast_to([np_, nsa, 2])
                    V.tensor_tensor(out=d_t, in0=g1, in1=g0, op=ALU.subtract)
                    V.tensor_tensor(out=d_t, in0=d_t, in1=wxv, op=ALU.mult)
                    V.tensor_tensor(out=d_o, in0=d_t, in1=g0, op=ALU.add)
                # y-lerp
                XL4 = XL[:, :, :].rearrange("p (s a) i -> p s a i", a=2)
                wyv = WYv[:, s_lo:s_hi].unsqueeze(2).broadcast_to(
                    [np_, ns, 32])
                sv = samp[:, 32 * s_lo:32 * s_hi] \
                    .rearrange("p (s i) -> p s i", s=ns)
                dv = XD[:, 0:ns, :]
                V.tensor_tensor(out=dv, in0=XL4[:, :, 1, :],
                                in1=XL4[:, :, 0, :], op=ALU.subtract)
                V.tensor_tensor(out=dv, in0=dv, in1=wyv, op=ALU.mult)
                V.tensor_tensor(out=sv, in0=dv, in1=XL4[:, :, 0, :],
                                op=ALU.add)

            sampF = pool.tile([128, 128], F32)     # [(b,c,k4)][(s,i)]
            XD0 = pool.tile([128, 4, 32], F32)
            XL0 = pool.tile([128, 4, 32], F32)
            XD1 = pool.tile([128, 4, 32], F32)
            XL1 = pool.tile([128, 4, 32], F32)
            xy_interp(tF, 128, WXT, WY4, sampF, XD0, XL0, 0, 2)
            xy_interp(tF, 128, WXT, WY4, sampF, XD1, XL1, 2, 4)
            sampM = pool.tile([64, 128], F32)      # [(b,k4)][(s,i)]
            XDM = pool.tile([64, 8, 32], F32)
            XLM = pool.tile([64, 8, 32], F32)
            xy_interp(tM, 64, WXT[0:64, :], WY4M, sampM, XDM, XLM, 0, 4)

            # ------- transpose + scale + grid + normalize (one PE pass) -----
            psF = psp.tile([128, 128], F32)
            T.matmul(psF[:, :], sampF[:, :], SXYDn, start=True, stop=False)
            T.matmul(psF[:, :], G6, GR6, start=False, stop=True)
            PQs = pool.tile([128, 128], F32)  # normalized dst [m=(s,i)][(b,c,t)]
            V.tensor_copy(PQs[:, :], psF[:, :])
            psM = psp.tile([128, 64], F32)
            T.transpose(psM[:, :], sampM[:, :], IDN)
            SM = pool.tile([128, 64], F32)    # mask sample [m][(b,t)]
            V.tensor_copy(SM[:, :], psM[:, :])

            # ---------------- D = [w, w*p, w*q, w*r2] ----------------
            D = pool.tile([128, 256], F32)    # [m][(t,q,b)]
            Dv = D[:, :].rearrange("p (t q b) -> p q t b", q=4, b=8)
            V.tensor_scalar(out=Dv[:, 0, :, :],
                            in0=SM[:, :].rearrange("p (b t) -> p t b", t=8),
                            scalar1=0.0, op0=ALU.max, scalar2=None)
            d12 = D[:, :].rearrange("p (t q b) -> p t q b", q=4, b=8)[:, :, 1:3, :]
            pq12 = PQs[:, :].rearrange("p (b c t) -> p t c b", t=8, b=8, c=2)
            wb2 = Dv[:, 0, :, :].unsqueeze(2).broadcast_to([128, 8, 2, 8])
            V.tensor_tensor(out=d12, in0=pq12, in1=wb2, op=ALU.mult)
            SQ = pool.tile([128, 128], F32)
            V.tensor_tensor(out=SQ[:, :], in0=PQs[:, :], in1=PQs[:, :],
                            op=ALU.mult)
            R2 = pool.tile([128, 64], F32)
            sq3 = SQ[:, :].rearrange("p (b c t) -> p c t b", t=8, b=8, c=2)
            V.tensor_tensor(out=R2[:, :].rearrange("p (t b) -> p t b", t=8),
                            in0=sq3[:, 0, :, :], in1=sq3[:, 1, :, :], op=ALU.add)
            V.tensor_tensor(out=Dv[:, 3, :, :],
                            in0=R2[:, :].rearrange("p (t b) -> p t b", t=8),
                            in1=Dv[:, 0, :, :], op=ALU.mult)

            # ------- moments: psMom[f, (q,b)] = sum_t C9_t^T D_t ----------
            psMom = psp.tile([9, 32], F32)
            for t in range(8):
                T.matmul(psMom[:, :], C9[:, 9 * t:9 * t + 9],
                         D[:, 32 * t:32 * t + 32], start=(t == 0),
                         stop=(t == 7))
            Msb = pool.tile([9, 32], F32)
            V.tensor_copy(Msb[:, :], psMom[:, :])
            # per-q PE transposes: [8 = batch, 9 = feature] tiles
            MQT = [pool.tile([8, 9], F32, name=f"MQT{q}") for q in range(4)]
            psQ = [psp.tile([8, 9], F32, name=f"psQ{q}") for q in range(4)]
            for q in range(4):
                T.transpose(psQ[q][:, :], Msb[:, 8 * q:8 * q + 8],
                            IDN[0:9, 0:9])
                V.tensor_copy(MQT[q][:, :], psQ[q][:, :])
            M0, Mp, Mq, Mr = MQT

            # ---------------- assemble AUG [8, 72] ----------------
            AUG = pool.tile([8, 72], F32)
            V.memset(AUG[:, :], 0.0)
            a33 = AUG[:, :].rearrange("p (r c) -> p r c", r=8)

            def v33(mt, tr=False):
                vv = mt[:, :].rearrange("p (r c) -> p r c", r=3)
                return vv.rearrange("p r c -> p c r") if tr else vv

            V.tensor_copy(a33[:, 0:3, 0:3], v33(M0))
            V.tensor_copy(a33[:, 3:6, 3:6], v33(M0))
            V.tensor_scalar(out=a33[:, 0:3, 6:8], in0=v33(Mp)[:, :, 0:2],
                            scalar1=-1.0, op0=ALU.mult, scalar2=None)
            V.tensor_scalar(out=a33[:, 3:6, 6:8], in0=v33(Mq)[:, :, 0:2],
                            scalar1=-1.0, op0=ALU.mult, scalar2=None)
            V.tensor_scalar(out=a33[:, 6:8, 0:3], in0=v33(Mp, tr=True)[:, 0:2, :],
                            scalar1=-1.0, op0=ALU.mult, scalar2=None)
            V.tensor_scalar(out=a33[:, 6:8, 3:6], in0=v33(Mq, tr=True)[:, 0:2, :],
                            scalar1=-1.0, op0=ALU.mult, scalar2=None)
            V.tensor_copy(a33[:, 6:8, 6:8], v33(Mr)[:, 0:2, 0:2])
            V.tensor_copy(AUG[:, 8:27:9], Mp[:, 6:9])
            V.tensor_copy(AUG[:, 35:54:9], Mq[:, 6:9])
            V.tensor_scalar(out=AUG[:, 62:72:9], in0=Mr[:, 6:8],
                            scalar1=-1.0, op0=ALU.mult, scalar2=None)
            V.tensor_scalar(out=AUG[:, 0:71:10], in0=AUG[:, 0:71:10],
                            scalar1=EPS, op0=ALU.add, scalar2=None)
            # support gate from AUG[2,2] = sum(w) + eps
            GT = pool.tile([8, 1], F32)
            V.tensor_scalar(out=GT[:, :], in0=AUG[:, 20:21],
                            scalar1=NPTS * 1e-4 + EPS, op0=ALU.is_gt,
                            scalar2=None)

            # ------- Gauss-Jordan (unnormalized rows) -------
            FC = pool.tile([8, 8], F32)
            PIV = pool.tile([8, 1], F32)
            DRC = pool.tile([8, 8], F32)
            U8 = pool.tile([8, 72], F32)
            for kk in range(8):
                w_ = 9 - kk
                # F[i] = a_ik / a_kk, with F[k] forced to 0 so row k survives
                V.reciprocal(PIV[:, :], AUG[:, 9 * kk + kk:9 * kk + kk + 1])
                V.tensor_scalar(out=FC[:, :], in0=AUG[:, kk:72:9],
                                scalar1=PIV[:, :], op0=ALU.mult, scalar2=None)
                V.memset(FC[:, kk:kk + 1], 0.0)
                fcol = FC[:, :].unsqueeze(2).broadcast_to([8, 8, w_])
                rkb = AUG[:, 9 * kk + kk:9 * kk + 9].unsqueeze(1) \
                    .broadcast_to([8, 8, w_])
                ucols = U8[:, :].rearrange("p (r c) -> p r c", r=8)[:, :, 0:w_]
                acols = AUG[:, :].rearrange("p (r c) -> p r c", r=8)[:, :, kk:9]
                V.tensor_tensor(out=ucols, in0=fcol, in1=rkb, op=ALU.mult)
                V.tensor_tensor(out=acols, in0=acols, in1=ucols, op=ALU.subtract)

            # ---------------- denormalize (immediate T) ----------------
            c_ = V.tensor_copy
            HN = pool.tile([8, 9], F32)
            V.reciprocal(DRC[:, :], AUG[:, 0:72:10])
            V.tensor_tensor(out=HN[:, 0:8], in0=AUG[:, 8:72:9],
                            in1=DRC[:, :], op=ALU.mult)
            V.memset(HN[:, 8:9], 1.0)
            s_c, mx, my, a_t = cc.s_c, cc.mx, cc.my, cc.a_t
            T1 = pool.tile([8, 9], F32)
            H1 = pool.tile([8, 9], F32)
            V.tensor_scalar(out=T1[:, 0:3], in0=HN[:, 0:3], scalar1=s_c,
                            op0=ALU.mult, scalar2=None)
            V.scalar_tensor_tensor(out=H1[:, 0:3], in0=HN[:, 6:9], scalar=mx,
                                   in1=T1[:, 0:3], op0=ALU.mult, op1=ALU.add)
            V.tensor_scalar(out=T1[:, 3:6], in0=HN[:, 3:6], scalar1=s_c,
                            op0=ALU.mult, scalar2=None)
            V.scalar_tensor_tensor(out=H1[:, 3:6], in0=HN[:, 6:9], scalar=my,
                                   in1=T1[:, 3:6], op0=ALU.mult, op1=ALU.add)
            c_(H1[:, 6:9], HN[:, 6:9])
            H2 = pool.tile([8, 9], F32)
            H1v = H1[:, :].rearrange("p (r c) -> p r c", r=3)
            H2v = H2[:, :].rearrange("p (r c) -> p r c", r=3)
            V.tensor_scalar(out=H2v[:, :, 0:2], in0=H1v[:, :, 0:2],
                            scalar1=a_t, op0=ALU.mult, scalar2=None)
            T2 = pool.tile([8, 3], F32)
            T3 = pool.tile([8, 3], F32)
            V.tensor_scalar(out=T2[:, :], in0=H1[:, 0:9:3], scalar1=-mx * a_t,
                            op0=ALU.mult, scalar2=None)
            V.scalar_tensor_tensor(out=T3[:, :], in0=H1[:, 1:9:3],
                                   scalar=-my * a_t, in1=T2[:, :],
                                   op0=ALU.mult, op1=ALU.add)
            V.tensor_tensor(out=H2[:, 2:9:3], in0=T3[:, :], in1=H1[:, 2:9:3],
                            op=ALU.add)
            # sign/scale fix: H /= (H22 + sign(H22)*1e-8)
            ISN = pool.tile([8, 1], F32)
            DEN = pool.tile([8, 1], F32)
            RECD = pool.tile([8, 1], F32)
            V.tensor_scalar(out=ISN[:, :], in0=H2[:, 8:9], scalar1=0.0,
                            op0=ALU.is_lt, scalar2=-2e-8, op1=ALU.mult)
            V.tensor_scalar(out=ISN[:, :], in0=ISN[:, :], scalar1=1e-8,
                            op0=ALU.add, scalar2=None)
            V.tensor_tensor(out=DEN[:, :], in0=H2[:, 8:9], in1=ISN[:, :],
                            op=ALU.add)
            V.reciprocal(RECD[:, :], DEN[:, :])
            V.tensor_scalar(out=H2[:, :], in0=H2[:, :], scalar1=RECD[:, :],
                            op0=ALU.mult, scalar2=None)
            # support gate
            IG = pool.tile([8, 1], F32)
            TI = pool.tile([8, 9], F32)
            OUTt = pool.tile([8, 9], F32)
            V.tensor_scalar(out=IG[:, :], in0=GT[:, :], scalar1=-1.0,
                            op0=ALU.mult, scalar2=1.0, op1=ALU.add)
            V.tensor_scalar(out=TI[:, :], in0=IEYE[:, :], scalar1=IG[:, :],
                            op0=ALU.mult, scalar2=None)
            V.scalar_tensor_tensor(out=OUTt[:, :], in0=H2[:, :], scalar=GT[:, :],
                                   in1=TI[:, :], op0=ALU.mult, op1=ALU.add)
            S.dma_start(Hout.ap().rearrange("b r c -> b (r c)"), OUTt[:, :])

    nc.compile()
    return nc


# ---------------------------------------------------------------------------
# host wrapper
# ---------------------------------------------------------------------------

_CACHE = {}


def _get(img_h, img_w):
    key = (int(img_h), int(img_w))
    if key not in _CACHE:
        cc = _Consts(*key)
        _CACHE[key] = (cc, _build_program(cc))
    return _CACHE[key]


def _in_maps(cc, flow, mask):
    flow = np.ascontiguousarray(flow, np.float32)
    mask = np.ascontiguousarray(mask, np.float32)
    return [{
        "flow": flow[c * BPC:(c + 1) * BPC],
        "mask": mask[c * BPC:(c + 1) * BPC],
        "CB": cc.CB,
    } for c in range(NCORES)]


def run(flow, mask, img_h, img_w, trace=False, **spmd_kwargs):
    cc, nc = _get(img_h, img_w)
    res = bass_utils.run_bass_kernel_spmd(
        nc, _in_maps(cc, flow, mask), list(range(NCORES)), trace=trace,
        **spmd_kwargs)
    out = np.concatenate([res.results[c]["H"] for c in range(NCORES)], axis=0)
    return out.astype(np.float32), res


def kernel(flow, mask, img_h, img_w):
    out, _ = run(flow, mask, img_h, img_w)
    return out
